# revision 3
# baseline (speedup 1.0000x reference)
"""Trainium2 Bass kernel for nn_BasisAffinityGAT.

Math (per reference):
    fused = concat(desc, nve) @ W_fuse + b_fuse                 [B,N,D]
    q_k   = fused @ W_q[k];  k_k = fused @ W_k[k]               [B,N,R]
    e_q[b,k,n] = lrelu(q).a_q[k];  e_k likewise
    logits = e_q[:,:,:,None] + e_k[:,:,None,:]; symmetrized
    alpha  = softmax(logits, -1);  ema update; bias_log = log(clip(ema'))

Key algebra (exact):
  * sym-logits[i,j] = 0.5*(s_i + s_j) with s = e_q + e_k, so the row
    softmax collapses: alpha[b,k,i,j] = softmax_j(0.5*s[b,k,:])[j] for
    every row i.
  * lrelu(x) = 0.6*x + 0.4*|x|  (slope 0.2), so
    0.5*s[k,n] = fused[n,:] @ wlin[:,k] + 0.2*(a_q[k].|q_T|) + 0.2*(a_k[k].|k_T|)
    with wlin[:,k] = 0.3*(W_q[k] @ a_q[k] + W_k[k] @ a_k[k]) precomputed.
  * bias_log is batch-independent: [K,N,N] content broadcast over B.

Sharding (8 cores, SPMD): core m owns batch b=m for alpha ([K,N,N] out)
and basis k=m for bias_log ([B,N,N] out, B identical copies written).
pbar (mean over B of alpha row-distributions) via one tiny ReduceScatter
of p [K,N]: core m receives row m = sum_b p_b[m,:].
"""

import sys

import numpy as np

if "/opt/trn_rl_repo" not in sys.path:
    sys.path.insert(0, "/opt/trn_rl_repo")

from contextlib import ExitStack

import concourse.bass as bass
import concourse.tile as tile
from concourse import bacc, mybir
from concourse.bass_utils import run_bass_kernel_spmd

B, N, D, K = 8, 512, 128, 8
R = D
MOM = 0.99
EPS = 1e-6
N_CORES = 8
F32 = mybir.dt.float32
AF = mybir.ActivationFunctionType
ALU = mybir.AluOpType


def build():
    """Build the SPMD per-core Bass program (identical on all 8 cores)."""
    nc = bacc.Bacc("TRN2", target_bir_lowering=False, debug=False,
                   num_devices=N_CORES)

    # ---- per-core external tensors -------------------------------------
    xT = nc.dram_tensor("xT", [2, D, N], F32, kind="ExternalInput")
    wfuse = nc.dram_tensor("wfuse", [2, D, D], F32, kind="ExternalInput")
    bfuse = nc.dram_tensor("bfuse", [D, 1], F32, kind="ExternalInput")
    wq = nc.dram_tensor("wq", [K, D, R], F32, kind="ExternalInput")
    wk = nc.dram_tensor("wk", [K, D, R], F32, kind="ExternalInput")
    aqz = nc.dram_tensor("aqz", [K, R, K], F32, kind="ExternalInput")
    akz = nc.dram_tensor("akz", [K, R, K], F32, kind="ExternalInput")
    wlin = nc.dram_tensor("wlin", [D, K], F32, kind="ExternalInput")
    ema = nc.dram_tensor("ema", [N, N], F32, kind="ExternalInput")
    ones1 = nc.dram_tensor("ones1", [1, D], F32, kind="ExternalInput")
    cones = nc.dram_tensor("cones", [1, D], F32, kind="ExternalInput")
    alpha = nc.dram_tensor("alpha", [K, N, N], F32, kind="ExternalOutput")
    biaso = nc.dram_tensor("bias", [B, N, N], F32, kind="ExternalOutput")

    with ExitStack() as ctx:
        tc = ctx.enter_context(tile.TileContext(nc))
        const = ctx.enter_context(tc.tile_pool(name="const", bufs=1))
        work = ctx.enter_context(tc.tile_pool(name="work", bufs=2))
        absp = ctx.enter_context(tc.tile_pool(name="absp", bufs=2 * K))
        psum = ctx.enter_context(tc.tile_pool(name="psum", bufs=1, space="PSUM"))
        dram = ctx.enter_context(tc.tile_pool(name="dram", bufs=1, space="DRAM"))

        # ---- input loads ------------------------------------------------
        xT_sb = const.tile([D, 2 * N], F32)
        nc.sync.dma_start(xT_sb[:, 0:N], xT[0])
        nc.sync.dma_start(xT_sb[:, N:2 * N], xT[1])
        wfuse_sb = const.tile([D, 2 * D], F32)
        nc.sync.dma_start(wfuse_sb[:].rearrange("d (h c) -> d h c", h=2),
                          wfuse.ap().rearrange("h d c -> d h c"))
        bfuse_sb = const.tile([D, 1], F32)
        nc.sync.dma_start(bfuse_sb[:], bfuse[:])
        wq_sb = const.tile([D, K * R], F32)
        nc.sync.dma_start(wq_sb[:].rearrange("d (k r) -> d k r", k=K),
                          wq.ap().rearrange("k d r -> d k r"))
        wk_sb = const.tile([D, K * R], F32)
        nc.sync.dma_start(wk_sb[:].rearrange("d (k r) -> d k r", k=K),
                          wk.ap().rearrange("k d r -> d k r"))
        aqz_sb = const.tile([R, K * K], F32)
        nc.sync.dma_start(aqz_sb[:].rearrange("r (k c) -> r k c", k=K),
                          aqz.ap().rearrange("k r c -> r k c"))
        akz_sb = const.tile([R, K * K], F32)
        nc.sync.dma_start(akz_sb[:].rearrange("r (k c) -> r k c", k=K),
                          akz.ap().rearrange("k r c -> r k c"))
        wlin_sb = const.tile([D, K], F32)
        nc.sync.dma_start(wlin_sb[:], wlin[:])
        ones1_sb = const.tile([1, D], F32)
        nc.sync.dma_start(ones1_sb[:], ones1[:])
        cones_sb = const.tile([1, D], F32)
        nc.sync.dma_start(cones_sb[:], cones[:])
        ema_sb = const.tile([128, 4 * N], F32)
        nc.sync.dma_start(ema_sb[:].rearrange("p (c n) -> p c n", c=4),
                          ema.ap().rearrange("(c p) n -> p c n", p=128))

        # ---- fused_T = W_fuse.T @ concat_T + b_fuse  [D, N] -------------
        psum_f = psum.tile([D, N], F32, tag="pf")
        nc.tensor.matmul(psum_f[:], wfuse_sb[:, 0:D], xT_sb[:, 0:N],
                         start=True, stop=False)
        nc.tensor.matmul(psum_f[:], wfuse_sb[:, D:2 * D], xT_sb[:, N:2 * N],
                         start=False, stop=True)
        fused_sb = const.tile([D, N], F32)
        nc.scalar.activation(fused_sb[:], psum_f[:], AF.Identity,
                             bias=bfuse_sb[:], scale=1.0)

        # ---- per-basis |q_T|, |k_T|  [D, N] each ------------------------
        abs_tiles = []
        for k in range(K):
            psum_q = psum.tile([D, N], F32, tag="proj", bufs=2)
            nc.tensor.matmul(psum_q[:], wq_sb[:, bass.ts(k, R)], fused_sb[:],
                             start=True, stop=True)
            absq = absp.tile([D, N], F32, tag="abs")
            nc.scalar.activation(absq[:], psum_q[:], AF.Abs)
            psum_k = psum.tile([D, N], F32, tag="proj", bufs=2)
            nc.tensor.matmul(psum_k[:], wk_sb[:, bass.ts(k, R)], fused_sb[:],
                             start=True, stop=True)
            absk = absp.tile([D, N], F32, tag="abs")
            nc.scalar.activation(absk[:], psum_k[:], AF.Abs)
            abs_tiles.append((absq, absk))

        # ---- s_half[k, n] accumulated in one PSUM tile [K, N] ----------
        psum_s = psum.tile([K, N], F32, tag="ps")
        nc.tensor.matmul(psum_s[:], wlin_sb[:], fused_sb[:],
                         start=True, stop=False)
        for k in range(K):
            absq, absk = abs_tiles[k]
            nc.tensor.matmul(psum_s[:], aqz_sb[:, bass.ts(k, K)], absq[:],
                             start=False, stop=False)
            nc.tensor.matmul(psum_s[:], akz_sb[:, bass.ts(k, K)], absk[:],
                             start=False, stop=(k == K - 1))

        # ---- row softmax over free dim: p[k, :] ------------------------
        mx = work.tile([K, 1], F32, bufs=1)
        nc.vector.reduce_max(mx[:], psum_s[:], axis=mybir.AxisListType.X)
        negmx = work.tile([K, 1], F32, bufs=1)
        nc.scalar.mul(negmx[:], mx[:], -1.0)
        expv = work.tile([K, N], F32, bufs=1)
        sume = work.tile([K, 1], F32, bufs=1)
        nc.scalar.activation(expv[:], psum_s[:], AF.Exp, bias=negmx[:],
                             scale=1.0, accum_out=sume[:])
        rsum = work.tile([K, 1], F32, bufs=1)
        nc.vector.reciprocal(rsum[:], sume[:])
        p_sb = work.tile([K, N], F32, bufs=1)
        nc.vector.tensor_scalar_mul(p_sb[:], expv[:], rsum[:])

        # ---- cross-core pbar: ReduceScatter(sum) of p [K,N] -> [1,N] ---
        cc_in = dram.tile([K, N], F32)
        nc.sync.dma_start(cc_in[:], p_sb[:])
        cc_out = dram.tile([1, N], F32)
        nc.gpsimd.collective_compute(
            "ReduceScatter", ALU.add,
            replica_groups=[list(range(N_CORES))],
            ins=[cc_in[:]], outs=[cc_out[:]],
        )
        pb_sb = work.tile([1, N], F32, bufs=1)
        nc.sync.dma_start(pb_sb[:], cc_out[:])

        # ---- alpha[k, i, :] = p[k, :] for all i ------------------------
        # gather p rows onto one partition, then PE-broadcast to 128
        p_flat = work.tile([1, K * N], F32, bufs=1)
        nc.sync.dma_start(
            p_flat[0:1, :].rearrange("o (k n) -> o k n", k=K), p_sb[:])
        rep_sb = const.tile([128, K * N], F32)
        for k in range(K):
            psum_rep = psum.tile([128, N], F32, tag="rep", bufs=2)
            nc.tensor.matmul(psum_rep[:], ones1_sb[:],
                             p_flat[0:1, bass.ts(k, N)], start=True, stop=True)
            nc.vector.tensor_copy(rep_sb[:, bass.ts(k, N)], psum_rep[:])
            src = rep_sb[:, bass.ts(k, N)].rearrange(
                "p (o n) -> p o n", o=1).broadcast_to([128, 4, N])
            dst = alpha[k].rearrange("(p i) j -> p i j", p=128)
            nc.sync.dma_start(dst, src)

        # ---- bias_log chunk-wise ---------------------------------------
        # pbs = (0.01/8/0.99) * sum_b p  broadcast to 128 partitions
        psum_pb = psum.tile([128, N], F32, tag="pb")
        nc.tensor.matmul(psum_pb[:], cones_sb[:], pb_sb[:], start=True, stop=True)
        bias_sb = const.tile([128, 4 * N], F32)
        for c in range(4):
            u = work.tile([128, N], F32, tag="u", bufs=2)
            nc.vector.tensor_add(u[:], ema_sb[:, bass.ts(c, N)], psum_pb[:])
            v = work.tile([128, N], F32, tag="v", bufs=2)
            nc.vector.tensor_scalar_max(v[:], u[:], EPS / MOM)
            nc.scalar.activation(bias_sb[:, bass.ts(c, N)], v[:], AF.Ln,
                                 scale=MOM)
            src = bias_sb[:, bass.ts(c, N)].rearrange(
                "p (o n) -> p o n", o=1).broadcast_to([128, B, N])
            dst = biaso.ap().rearrange("b (c p) j -> c p b j", c=4)[c]
            nc.sync.dma_start(dst, src)

    nc.compile()
    return nc


_NC_CACHE = None


def _get_nc():
    global _NC_CACHE
    if _NC_CACHE is None:
        _NC_CACHE = build()
    return _NC_CACHE


def make_in_maps(desc_embeddings, name_value_embeddings, W_fuse, b_fuse,
                 W_q, W_k, a, alpha_ema):
    """Host-side sharding / weight prep -> per-core input dicts."""
    desc = np.asarray(desc_embeddings, np.float32)
    nve = np.asarray(name_value_embeddings, np.float32)
    W_fuse = np.asarray(W_fuse, np.float32)
    b_fuse = np.asarray(b_fuse, np.float32)
    W_q = np.asarray(W_q, np.float32)
    W_k = np.asarray(W_k, np.float32)
    a = np.asarray(a, np.float32)
    alpha_ema = np.asarray(alpha_ema, np.float32)

    a_q = a[:, :R, 0]                      # [K,R]
    a_k = a[:, R:, 0]                      # [K,R]
    aqz = np.zeros((K, R, K), np.float32)
    akz = np.zeros((K, R, K), np.float32)
    for k in range(K):
        aqz[k, :, k] = 0.2 * a_q[k]
        akz[k, :, k] = 0.2 * a_k[k]
    wlin = 0.3 * (np.einsum("kdr,kr->dk", W_q, a_q)
                  + np.einsum("kdr,kr->dk", W_k, a_k)).astype(np.float32)
    wfuse_stack = np.ascontiguousarray(W_fuse.reshape(2, D, D))
    bfuse_col = np.ascontiguousarray(b_fuse.reshape(D, 1))
    ones1 = np.ones((1, D), np.float32)
    cones = np.full((1, D), 0.01 / B / MOM, np.float32)

    shared = dict(wfuse=wfuse_stack, bfuse=bfuse_col,
                  wq=np.ascontiguousarray(W_q), wk=np.ascontiguousarray(W_k),
                  aqz=aqz, akz=akz, wlin=np.ascontiguousarray(wlin),
                  ones1=ones1, cones=cones)
    in_maps = []
    for m in range(N_CORES):
        xT = np.ascontiguousarray(
            np.stack([desc[m].T, nve[m].T], axis=0))
        in_maps.append(dict(shared, xT=xT,
                            ema=np.ascontiguousarray(alpha_ema[m])))
    return in_maps


def gather(results):
    alpha_full = np.stack([r["alpha"] for r in results], axis=0)
    bias_full = np.stack([r["bias"] for r in results], axis=1)
    return bias_full, alpha_full


def kernel(**inputs):
    nc = _get_nc()
    in_maps = make_in_maps(**inputs)
    res = run_bass_kernel_spmd(nc, in_maps, list(range(N_CORES)))
    return gather(res.results)


# revision 7
# speedup vs baseline: 1.3285x; 1.3285x over previous
"""Trainium2 Bass kernel for nn_BasisAffinityGAT (B=8, N=512, D=R=128, K=8).

Math (matches reference.py):
    fused = concat(desc, nve) @ W_fuse + b_fuse                 [B,N,D]
    q = fused @ W_q[k];  kk = fused @ W_k[k]                    per basis
    e_q[b,k,n] = lrelu(q).a_q[k];  e_k likewise
    logits = e_q[:,:,:,None] + e_k[:,:,None,:], symmetrized
    alpha  = softmax(logits, -1); ema update; bias_log = log(clip(ema'))

Exact algebra used:
  * sym-logits[i,j] = 0.5*(s_i + s_j) with s = e_q + e_k, so the row
    softmax collapses: alpha[b,k,i,j] = softmax_j(0.5*s[b,k,:])[j],
    independent of i.
  * lrelu(x) = 0.6*x + 0.4*|x| (slope 0.2), so
    0.5*s[b,k,n] = fused[b,n,:] @ wlin[:,k]
                   + 0.2*(a_q[k] . |q_T|) + 0.2*(a_k[k] . |k_T|)
    with wlin[:,k] = 0.3*(W_q[k] @ a_q[k] + W_k[k] @ a_k[k]) host-folded.
  * bias_log content is batch-independent ([K,N,N] broadcast over B).

Sharding (8 cores, SPMD, zero cross-core communication): core m owns
basis k=m for ALL batches.  It computes p[b,:] = softmax_j(0.5*s[b,m,:])
for b=0..7, writes alpha[:,m,:,:] (rows of each [N,N] block all equal
p[b,:]) and bias_log[:,m,:,:] (B identical copies).  pbar (batch mean)
is a local partition-sum matmul — the ncfw collective (~78us launch
latency on this runtime) is avoided entirely.
"""

import sys

import numpy as np

if "/opt/trn_rl_repo" not in sys.path:
    sys.path.insert(0, "/opt/trn_rl_repo")

from contextlib import ExitStack

import concourse.bass as bass
import concourse.tile as tile
from concourse import bacc, mybir
from concourse.bass_utils import run_bass_kernel_spmd

B, N, D, K = 8, 512, 128, 8
R = D
MOM = 0.99
EPS = 1e-6
N_CORES = 8
F32 = mybir.dt.float32
F32R = mybir.dt.float32r
AF = mybir.ActivationFunctionType
ALU = mybir.AluOpType


def build():
    """Build the SPMD per-core Bass program (identical on all 8 cores)."""
    nc = bacc.Bacc("TRN2", target_bir_lowering=False, debug=False,
                   num_devices=N_CORES)

    # ---- per-core external tensors -------------------------------------
    # xTall[b,h,d,n]: h=0 desc[b].T, h=1 nve[b].T  (same array on all cores)
    xTall = nc.dram_tensor("xTall", [B, 2, D, N], F32, kind="ExternalInput")
    wfuse = nc.dram_tensor("wfuse", [2, D, D], F32, kind="ExternalInput")
    bfuse = nc.dram_tensor("bfuse", [D, 1], F32, kind="ExternalInput")
    wq = nc.dram_tensor("wq", [D, R], F32R, kind="ExternalInput")   # W_q[m]
    wk = nc.dram_tensor("wk", [D, R], F32R, kind="ExternalInput")   # W_k[m]
    aqzB = nc.dram_tensor("aqzB", [B, R, B], F32R, kind="ExternalInput")
    akzB = nc.dram_tensor("akzB", [B, R, B], F32R, kind="ExternalInput")
    wlinB = nc.dram_tensor("wlinB", [B, D, B], F32R, kind="ExternalInput")
    ema = nc.dram_tensor("ema", [N, N], F32, kind="ExternalInput")  # [m]
    alpha = nc.dram_tensor("alpha", [B, N, N], F32, kind="ExternalOutput")
    biaso = nc.dram_tensor("bias", [B, N, N], F32, kind="ExternalOutput")

    with ExitStack() as ctx:
        tc = ctx.enter_context(tile.TileContext(nc))
        const = ctx.enter_context(tc.tile_pool(name="const", bufs=1))
        work = ctx.enter_context(tc.tile_pool(name="work", bufs=2))
        absp = ctx.enter_context(tc.tile_pool(name="absp", bufs=2 * B))
        psum = ctx.enter_context(tc.tile_pool(name="psum", bufs=1, space="PSUM"))

        # ---- small constant loads --------------------------------------
        wfuse_sb = const.tile([D, 2 * D], F32)
        nc.sync.dma_start(wfuse_sb[:].rearrange("d (h c) -> d h c", h=2),
                          wfuse.ap().rearrange("h d c -> d h c"))
        bfuse_sb = const.tile([D, 1], F32)
        nc.sync.dma_start(bfuse_sb[:], bfuse[:])
        wq_sb = const.tile([D, R], F32R)
        nc.sync.dma_start(wq_sb[:], wq[:])
        wk_sb = const.tile([D, R], F32R)
        nc.sync.dma_start(wk_sb[:], wk[:])
        aqz_sb = const.tile([R, B * B], F32R)
        nc.sync.dma_start(aqz_sb[:].rearrange("r (b c) -> r b c", b=B),
                          aqzB.ap().rearrange("b r c -> r b c"))
        akz_sb = const.tile([R, B * B], F32R)
        nc.sync.dma_start(akz_sb[:].rearrange("r (b c) -> r b c", b=B),
                          akzB.ap().rearrange("b r c -> r b c"))
        wlin_sb = const.tile([D, B * B], F32R)
        nc.sync.dma_start(wlin_sb[:].rearrange("d (b c) -> d b c", b=B),
                          wlinB.ap().rearrange("b d c -> d b c"))
        ema_sb = const.tile([128, 4 * N], F32)
        nc.sync.dma_start(ema_sb[:].rearrange("p (c n) -> p c n", c=4),
                          ema.ap().rearrange("(c p) n -> p c n", p=128))
        ones1_sb = const.tile([1, D], F32)
        nc.vector.memset(ones1_sb[:], 1.0)
        # cones8: [8,1] column of (0.01/8/0.99) for the pbar partition-sum
        cones8_sb = const.tile([B, 1], F32)
        nc.vector.memset(cones8_sb[:], 0.01 / B / MOM)

        # ---- per-batch: fused_T, |q_T|, |k_T| --------------------------
        abs_tiles = []
        fused_tiles = []
        for b in range(B):
            xb = work.tile([D, 2 * N], F32, tag="xb", bufs=3)
            nc.sync.dma_start(
                xb[:].rearrange("d (h n) -> d h n", h=2),
                xTall[b].rearrange("h d n -> d h n"))
            psum_f = psum.tile([D, N], F32, tag="mm", bufs=3)
            nc.tensor.matmul(psum_f[:], wfuse_sb[:, 0:D], xb[:, 0:N],
                             start=True, stop=False)
            nc.tensor.matmul(psum_f[:], wfuse_sb[:, D:2 * D], xb[:, N:2 * N],
                             start=False, stop=True)
            fused_sb = absp.tile([D, N], F32R, tag="fused", bufs=B)
            nc.scalar.activation(fused_sb[:], psum_f[:], AF.Identity,
                                 bias=bfuse_sb[:], scale=1.0)
            psum_q = psum.tile([D, N], F32, tag="mm", bufs=3)
            nc.tensor.matmul(psum_q[:], wq_sb[:],
                             fused_sb[:], start=True, stop=True)
            absq = absp.tile([D, N], F32R, tag="abs")
            nc.scalar.activation(absq[:], psum_q[:], AF.Abs)
            psum_k = psum.tile([D, N], F32, tag="mm", bufs=3)
            nc.tensor.matmul(psum_k[:], wk_sb[:],
                             fused_sb[:], start=True, stop=True)
            absk = absp.tile([D, N], F32R, tag="abs")
            nc.scalar.activation(absk[:], psum_k[:], AF.Abs)
            abs_tiles.append((absq, absk))
            fused_tiles.append(fused_sb)

        # ---- s_half[b, n] accumulated in one PSUM tile [B, N] ----------
        psum_s = psum.tile([B, N], F32, tag="ps")
        for b in range(B):
            absq, absk = abs_tiles[b]
            nc.tensor.matmul(psum_s[:], wlin_sb[:, bass.ts(b, B)],
                             fused_tiles[b][:],
                             start=(b == 0), stop=False)
            nc.tensor.matmul(psum_s[:], aqz_sb[:, bass.ts(b, B)],
                             absq[:], start=False, stop=False)
            nc.tensor.matmul(psum_s[:], akz_sb[:, bass.ts(b, B)],
                             absk[:], start=False,
                             stop=(b == B - 1))

        # ---- row softmax over free dim: p[b, :] ------------------------
        mx = work.tile([B, 1], F32, bufs=1)
        nc.vector.reduce_max(mx[:], psum_s[:], axis=mybir.AxisListType.X)
        negmx = work.tile([B, 1], F32, bufs=1)
        nc.scalar.mul(negmx[:], mx[:], -1.0)
        expv = work.tile([B, N], F32, bufs=1)
        sume = work.tile([B, 1], F32, bufs=1)
        nc.scalar.activation(expv[:], psum_s[:], AF.Exp, bias=negmx[:],
                             scale=1.0, accum_out=sume[:])
        rsum = work.tile([B, 1], F32, bufs=1)
        nc.vector.reciprocal(rsum[:], sume[:])
        p_sb = work.tile([B, N], F32, bufs=1)
        nc.vector.tensor_scalar_mul(p_sb[:], expv[:], rsum[:])

        # ---- alpha[b, i, :] = p[b, :] for all i ------------------------
        p_flat = work.tile([1, B * N], F32, bufs=1)
        nc.sync.dma_start(
            p_flat[0:1, :].rearrange("o (b n) -> o b n", b=B), p_sb[:])
        rep_sb = const.tile([128, B * N], F32)
        for b in range(B):
            psum_rep = psum.tile([128, N], F32, tag="rep", bufs=2)
            nc.tensor.matmul(psum_rep[:], ones1_sb[:],
                             p_flat[0:1, bass.ts(b, N)], start=True, stop=True)
            nc.vector.tensor_copy(rep_sb[:, bass.ts(b, N)], psum_rep[:])
            src = rep_sb[:, bass.ts(b, N)].rearrange(
                "p (o n) -> p o n", o=1).broadcast_to([128, 4, N])
            dst = alpha[b].rearrange("(p i) j -> p i j", p=128)
            nc.sync.dma_start(dst, src)

        # ---- bias_log: pbar is LOCAL (partition-sum over batches) ------
        psum_pb1 = psum.tile([1, N], F32, tag="pb1")
        nc.tensor.matmul(psum_pb1[:], cones8_sb[:], p_sb[:],
                         start=True, stop=True)
        pb_sb = work.tile([1, N], F32, bufs=1)
        nc.vector.tensor_copy(pb_sb[:], psum_pb1[:])
        psum_pb = psum.tile([128, N], F32, tag="pb")
        nc.tensor.matmul(psum_pb[:], ones1_sb[:], pb_sb[:],
                         start=True, stop=True)
        bias_sb = const.tile([128, 4 * N], F32)
        for c in range(4):
            u = work.tile([128, N], F32, tag="u", bufs=2)
            nc.vector.tensor_add(u[:], ema_sb[:, bass.ts(c, N)], psum_pb[:])
            v = work.tile([128, N], F32, tag="v", bufs=2)
            nc.vector.tensor_scalar_max(v[:], u[:], EPS / MOM)
            nc.scalar.activation(bias_sb[:, bass.ts(c, N)], v[:], AF.Ln,
                                 scale=MOM)
            src = bias_sb[:, bass.ts(c, N)].rearrange(
                "p (o n) -> p o n", o=1).broadcast_to([128, B, N])
            dst = biaso.ap().rearrange("b (c p) j -> c p b j", c=4)[c]
            nc.sync.dma_start(dst, src)

    nc.compile()
    return nc


_NC_CACHE = None


def _get_nc():
    global _NC_CACHE
    if _NC_CACHE is None:
        _NC_CACHE = build()
    return _NC_CACHE


def make_in_maps(desc_embeddings, name_value_embeddings, W_fuse, b_fuse,
                 W_q, W_k, a, alpha_ema):
    """Host-side sharding / weight prep -> per-core input dicts."""
    desc = np.asarray(desc_embeddings, np.float32)
    nve = np.asarray(name_value_embeddings, np.float32)
    W_fuse = np.asarray(W_fuse, np.float32)
    b_fuse = np.asarray(b_fuse, np.float32)
    W_q = np.asarray(W_q, np.float32)
    W_k = np.asarray(W_k, np.float32)
    a = np.asarray(a, np.float32)
    alpha_ema = np.asarray(alpha_ema, np.float32)

    a_q = a[:, :R, 0]                      # [K,R]
    a_k = a[:, R:, 0]                      # [K,R]
    wlin = 0.3 * (np.einsum("kdr,kr->kd", W_q, a_q)
                  + np.einsum("kdr,kr->kd", W_k, a_k))  # [K,D]

    # xTall[b] = [desc[b].T, nve[b].T] — shared across cores
    xTall = np.ascontiguousarray(
        np.stack([np.stack([desc[b].T, nve[b].T], axis=0)
                  for b in range(B)], axis=0))
    wfuse_stack = np.ascontiguousarray(W_fuse.reshape(2, D, D))
    bfuse_col = np.ascontiguousarray(b_fuse.reshape(D, 1))

    shared = dict(xTall=xTall, wfuse=wfuse_stack, bfuse=bfuse_col)
    in_maps = []
    for m in range(N_CORES):
        aqzB = np.zeros((B, R, B), np.float32)
        akzB = np.zeros((B, R, B), np.float32)
        wlinB = np.zeros((B, D, B), np.float32)
        for b in range(B):
            aqzB[b, :, b] = 0.2 * a_q[m]
            akzB[b, :, b] = 0.2 * a_k[m]
            wlinB[b, :, b] = wlin[m]
        in_maps.append(dict(
            shared,
            wq=np.ascontiguousarray(W_q[m]),
            wk=np.ascontiguousarray(W_k[m]),
            aqzB=aqzB, akzB=akzB, wlinB=wlinB,
            ema=np.ascontiguousarray(alpha_ema[m])))
    return in_maps


def gather(results):
    alpha_full = np.stack([r["alpha"] for r in results], axis=1)
    bias_full = np.stack([r["bias"] for r in results], axis=1)
    return bias_full, alpha_full


def kernel(**inputs):
    nc = _get_nc()
    in_maps = make_in_maps(**inputs)
    res = run_bass_kernel_spmd(nc, in_maps, list(range(N_CORES)))
    return gather(res.results)


# revision 8
# speedup vs baseline: 1.3546x; 1.0196x over previous
"""Trainium2 Bass kernel for nn_BasisAffinityGAT (B=8, N=512, D=R=128, K=8).

Math (matches reference.py):
    fused = concat(desc, nve) @ W_fuse + b_fuse                 [B,N,D]
    q = fused @ W_q[k];  kk = fused @ W_k[k]                    per basis
    e_q[b,k,n] = lrelu(q).a_q[k];  e_k likewise
    logits = e_q[:,:,:,None] + e_k[:,:,None,:], symmetrized
    alpha  = softmax(logits, -1); ema update; bias_log = log(clip(ema'))

Exact algebra used:
  * sym-logits[i,j] = 0.5*(s_i + s_j) with s = e_q + e_k, so the row
    softmax collapses: alpha[b,k,i,j] = softmax_j(0.5*s[b,k,:])[j],
    independent of i.
  * lrelu(x) = 0.6*x + 0.4*|x| (slope 0.2), so
    0.5*s[b,k,n] = fused[b,n,:] @ wlin[:,k]
                   + 0.2*(a_q[k] . |q_T|) + 0.2*(a_k[k] . |k_T|)
    with wlin[:,k] = 0.3*(W_q[k] @ a_q[k] + W_k[k] @ a_k[k]) host-folded.
  * bias_log content is batch-independent ([K,N,N] broadcast over B).

Sharding (8 cores, SPMD, zero cross-core communication): core m owns
basis k=m for ALL batches; pbar (batch mean) is a local partition-sum.
Batches are processed in two groups of 4 so the first half of the
alpha output DMA (the dominant cost) overlaps the second half's
compute.  PE matmuls run fp32r (fp22 multiplies, fp32 accumulate)
except the p-broadcasts, which stay exact fp32.
"""

import sys

import numpy as np

if "/opt/trn_rl_repo" not in sys.path:
    sys.path.insert(0, "/opt/trn_rl_repo")

from contextlib import ExitStack

import concourse.bass as bass
import concourse.tile as tile
from concourse import bacc, mybir
from concourse.bass_utils import run_bass_kernel_spmd

B, N, D, K = 8, 512, 128, 8
R = D
MOM = 0.99
EPS = 1e-6
N_CORES = 8
G = 4                      # batches per softmax group
F32 = mybir.dt.float32
F32R = mybir.dt.float32r
AF = mybir.ActivationFunctionType
ALU = mybir.AluOpType


def build():
    """Build the SPMD per-core Bass program (identical on all 8 cores)."""
    nc = bacc.Bacc("TRN2", target_bir_lowering=False, debug=False,
                   num_devices=N_CORES)

    # ---- per-core external tensors -------------------------------------
    # xTall[b,h,d,n]: h=0 desc[b].T, h=1 nve[b].T  (same array on all cores)
    xTall = nc.dram_tensor("xTall", [B, 2, D, N], F32R, kind="ExternalInput")
    wfuse = nc.dram_tensor("wfuse", [2, D, D], F32R, kind="ExternalInput")
    bfuse = nc.dram_tensor("bfuse", [D, 1], F32, kind="ExternalInput")
    wq = nc.dram_tensor("wq", [D, R], F32R, kind="ExternalInput")   # W_q[m]
    wk = nc.dram_tensor("wk", [D, R], F32R, kind="ExternalInput")   # W_k[m]
    aqzB = nc.dram_tensor("aqzB", [B, R, B], F32R, kind="ExternalInput")
    akzB = nc.dram_tensor("akzB", [B, R, B], F32R, kind="ExternalInput")
    wlinB = nc.dram_tensor("wlinB", [B, D, B], F32R, kind="ExternalInput")
    ema = nc.dram_tensor("ema", [N, N], F32, kind="ExternalInput")  # [m]
    alpha = nc.dram_tensor("alpha", [B, N, N], F32, kind="ExternalOutput")
    biaso = nc.dram_tensor("bias", [B, N, N], F32, kind="ExternalOutput")

    with ExitStack() as ctx:
        tc = ctx.enter_context(tile.TileContext(nc))
        const = ctx.enter_context(tc.tile_pool(name="const", bufs=1))
        work = ctx.enter_context(tc.tile_pool(name="work", bufs=2))
        absp = ctx.enter_context(tc.tile_pool(name="absp", bufs=4))
        psum = ctx.enter_context(tc.tile_pool(name="psum", bufs=1, space="PSUM"))

        # ---- small constant loads --------------------------------------
        wfuse_sb = const.tile([D, 2 * D], F32R)
        nc.sync.dma_start(wfuse_sb[:].rearrange("d (h c) -> d h c", h=2),
                          wfuse.ap().rearrange("h d c -> d h c"))
        bfuse_sb = const.tile([D, 1], F32)
        nc.sync.dma_start(bfuse_sb[:], bfuse[:])
        wq_sb = const.tile([D, R], F32R)
        nc.sync.dma_start(wq_sb[:], wq[:])
        wk_sb = const.tile([D, R], F32R)
        nc.sync.dma_start(wk_sb[:], wk[:])
        aqz_sb = const.tile([R, B * B], F32R)
        nc.sync.dma_start(aqz_sb[:].rearrange("r (b c) -> r b c", b=B),
                          aqzB.ap().rearrange("b r c -> r b c"))
        akz_sb = const.tile([R, B * B], F32R)
        nc.sync.dma_start(akz_sb[:].rearrange("r (b c) -> r b c", b=B),
                          akzB.ap().rearrange("b r c -> r b c"))
        wlin_sb = const.tile([D, B * B], F32R)
        nc.sync.dma_start(wlin_sb[:].rearrange("d (b c) -> d b c", b=B),
                          wlinB.ap().rearrange("b d c -> d b c"))
        ones1_sb = const.tile([1, D], F32)
        nc.vector.memset(ones1_sb[:], 1.0)
        # cones4: [4,1] column of (0.01/8/0.99) for the pbar partition-sum
        cones4_sb = const.tile([G, 1], F32)
        nc.vector.memset(cones4_sb[:], 0.01 / B / MOM)

        rep_sb = const.tile([128, B * N], F32)
        p_tiles = []

        for g in range(2):
            psum_sg = psum.tile([G, N], F32, tag=f"ps{g}")
            for rel in range(G):
                b = g * G + rel
                xb = work.tile([D, 2 * N], F32R, tag="xb", bufs=3)
                nc.sync.dma_start(
                    xb[:].rearrange("d (h n) -> d h n", h=2),
                    xTall[b].rearrange("h d n -> d h n"))
                psum_f = psum.tile([D, N], F32, tag="mm", bufs=2)
                nc.tensor.matmul(psum_f[:], wfuse_sb[:, 0:D], xb[:, 0:N],
                                 start=True, stop=False)
                nc.tensor.matmul(psum_f[:], wfuse_sb[:, D:2 * D],
                                 xb[:, N:2 * N], start=False, stop=True)
                fused_sb = absp.tile([D, N], F32R, tag="fused", bufs=2)
                nc.scalar.activation(fused_sb[:], psum_f[:], AF.Identity,
                                     bias=bfuse_sb[:], scale=1.0)
                lo = b * B + g * G          # lhsT col window [lo, lo+G)
                nc.tensor.matmul(psum_sg[:], wlin_sb[:, lo:lo + G],
                                 fused_sb[:], start=(rel == 0), stop=False,
                                 skip_group_check=True)
                psum_q = psum.tile([D, N], F32, tag="mm", bufs=2)
                nc.tensor.matmul(psum_q[:], wq_sb[:], fused_sb[:],
                                 start=True, stop=True)
                absq = absp.tile([D, N], F32R, tag="abs", bufs=2)
                nc.scalar.activation(absq[:], psum_q[:], AF.Abs)
                nc.tensor.matmul(psum_sg[:], aqz_sb[:, lo:lo + G],
                                 absq[:], start=False, stop=False,
                                 skip_group_check=True)
                psum_k = psum.tile([D, N], F32, tag="mm", bufs=2)
                nc.tensor.matmul(psum_k[:], wk_sb[:], fused_sb[:],
                                 start=True, stop=True)
                absk = absp.tile([D, N], F32R, tag="abs", bufs=2)
                nc.scalar.activation(absk[:], psum_k[:], AF.Abs)
                nc.tensor.matmul(psum_sg[:], akz_sb[:, lo:lo + G],
                                 absk[:], start=False, stop=(rel == G - 1),
                                 skip_group_check=True)

            # ---- row softmax over free dim: p[rel, :] ------------------
            mx = work.tile([G, 1], F32, tag=f"mx{g}", bufs=1)
            nc.vector.reduce_max(mx[:], psum_sg[:], axis=mybir.AxisListType.X)
            negmx = work.tile([G, 1], F32, tag=f"nmx{g}", bufs=1)
            nc.scalar.mul(negmx[:], mx[:], -1.0)
            expv = work.tile([G, N], F32, tag=f"ex{g}", bufs=1)
            sume = work.tile([G, 1], F32, tag=f"se{g}", bufs=1)
            nc.scalar.activation(expv[:], psum_sg[:], AF.Exp, bias=negmx[:],
                                 scale=1.0, accum_out=sume[:])
            rsum = work.tile([G, 1], F32, tag=f"rs{g}", bufs=1)
            nc.vector.reciprocal(rsum[:], sume[:])
            p_sb = work.tile([G, N], F32, tag=f"p{g}", bufs=1)
            nc.vector.tensor_scalar_mul(p_sb[:], expv[:], rsum[:])
            p_tiles.append(p_sb)

            # ---- alpha[b, i, :] = p[rel, :] for all i ------------------
            p_flat = work.tile([1, G * N], F32, tag=f"pf{g}", bufs=1)
            nc.sync.dma_start(
                p_flat[0:1, :].rearrange("o (r n) -> o r n", r=G), p_sb[:])
            for rel in range(G):
                b = g * G + rel
                psum_rep = psum.tile([128, N], F32, tag="rep", bufs=2)
                nc.tensor.matmul(psum_rep[:], ones1_sb[:],
                                 p_flat[0:1, bass.ts(rel, N)],
                                 start=True, stop=True)
                nc.vector.tensor_copy(rep_sb[:, bass.ts(b, N)], psum_rep[:])
                src = rep_sb[:, bass.ts(b, N)].rearrange(
                    "p (o n) -> p o n", o=1).broadcast_to([128, 4, N])
                dst = alpha[b].rearrange("(p i) j -> p i j", p=128)
                nc.sync.dma_start(dst, src)

        # ---- bias_log: pbar is LOCAL (partition-sum over batches) ------
        ema_sb = const.tile([128, 4 * N], F32)
        nc.sync.dma_start(ema_sb[:].rearrange("p (c n) -> p c n", c=4),
                          ema.ap().rearrange("(c p) n -> p c n", p=128))
        psum_pb1 = psum.tile([1, N], F32, tag="pb1")
        nc.tensor.matmul(psum_pb1[:], cones4_sb[:], p_tiles[0][:],
                         start=True, stop=False)
        nc.tensor.matmul(psum_pb1[:], cones4_sb[:], p_tiles[1][:],
                         start=False, stop=True)
        pb_sb = work.tile([1, N], F32, bufs=1)
        nc.vector.tensor_copy(pb_sb[:], psum_pb1[:])
        psum_pb = psum.tile([128, N], F32, tag="pb")
        nc.tensor.matmul(psum_pb[:], ones1_sb[:], pb_sb[:],
                         start=True, stop=True)
        bias_sb = const.tile([128, 4 * N], F32)
        for c in range(4):
            u = work.tile([128, N], F32, tag="u", bufs=2)
            nc.vector.tensor_add(u[:], ema_sb[:, bass.ts(c, N)], psum_pb[:])
            v = work.tile([128, N], F32, tag="v", bufs=2)
            nc.vector.tensor_scalar_max(v[:], u[:], EPS / MOM)
            nc.scalar.activation(bias_sb[:, bass.ts(c, N)], v[:], AF.Ln,
                                 scale=MOM)
            src = bias_sb[:, bass.ts(c, N)].rearrange(
                "p (o n) -> p o n", o=1).broadcast_to([128, B, N])
            dst = biaso.ap().rearrange("b (c p) j -> c p b j", c=4)[c]
            nc.sync.dma_start(dst, src)

    nc.compile()
    return nc


_NC_CACHE = None


def _get_nc():
    global _NC_CACHE
    if _NC_CACHE is None:
        _NC_CACHE = build()
    return _NC_CACHE


def make_in_maps(desc_embeddings, name_value_embeddings, W_fuse, b_fuse,
                 W_q, W_k, a, alpha_ema):
    """Host-side sharding / weight prep -> per-core input dicts."""
    desc = np.asarray(desc_embeddings, np.float32)
    nve = np.asarray(name_value_embeddings, np.float32)
    W_fuse = np.asarray(W_fuse, np.float32)
    b_fuse = np.asarray(b_fuse, np.float32)
    W_q = np.asarray(W_q, np.float32)
    W_k = np.asarray(W_k, np.float32)
    a = np.asarray(a, np.float32)
    alpha_ema = np.asarray(alpha_ema, np.float32)

    a_q = a[:, :R, 0]                      # [K,R]
    a_k = a[:, R:, 0]                      # [K,R]
    wlin = 0.3 * (np.einsum("kdr,kr->kd", W_q, a_q)
                  + np.einsum("kdr,kr->kd", W_k, a_k))  # [K,D]

    # xTall[b] = [desc[b].T, nve[b].T] — shared across cores
    xTall = np.ascontiguousarray(
        np.stack([np.stack([desc[b].T, nve[b].T], axis=0)
                  for b in range(B)], axis=0))
    wfuse_stack = np.ascontiguousarray(W_fuse.reshape(2, D, D))
    bfuse_col = np.ascontiguousarray(b_fuse.reshape(D, 1))

    shared = dict(xTall=xTall, wfuse=wfuse_stack, bfuse=bfuse_col)
    in_maps = []
    for m in range(N_CORES):
        aqzB = np.zeros((B, R, B), np.float32)
        akzB = np.zeros((B, R, B), np.float32)
        wlinB = np.zeros((B, D, B), np.float32)
        for b in range(B):
            aqzB[b, :, b] = 0.2 * a_q[m]
            akzB[b, :, b] = 0.2 * a_k[m]
            wlinB[b, :, b] = wlin[m]
        in_maps.append(dict(
            shared,
            wq=np.ascontiguousarray(W_q[m]),
            wk=np.ascontiguousarray(W_k[m]),
            aqzB=aqzB, akzB=akzB, wlinB=wlinB,
            ema=np.ascontiguousarray(alpha_ema[m])))
    return in_maps


def gather(results):
    alpha_full = np.stack([r["alpha"] for r in results], axis=1)
    bias_full = np.stack([r["bias"] for r in results], axis=1)
    return bias_full, alpha_full


def kernel(**inputs):
    nc = _get_nc()
    in_maps = make_in_maps(**inputs)
    res = run_bass_kernel_spmd(nc, in_maps, list(range(N_CORES)))
    return gather(res.results)


# revision 10
# speedup vs baseline: 1.4431x; 1.0653x over previous
"""Trainium2 Bass kernel for nn_BasisAffinityGAT (B=8, N=512, D=R=128, K=8).

Math (matches reference.py):
    fused = concat(desc, nve) @ W_fuse + b_fuse                 [B,N,D]
    q = fused @ W_q[k];  kk = fused @ W_k[k]                    per basis
    e_q[b,k,n] = lrelu(q).a_q[k];  e_k likewise
    logits = e_q[:,:,:,None] + e_k[:,:,None,:], symmetrized
    alpha  = softmax(logits, -1); ema update; bias_log = log(clip(ema'))

Exact algebra used:
  * sym-logits[i,j] = 0.5*(s_i + s_j) with s = e_q + e_k, so the row
    softmax collapses: alpha[b,k,i,j] = softmax_j(0.5*s[b,k,:])[j],
    independent of i.
  * lrelu(x) = 0.6*x + 0.4*|x| (slope 0.2), so
    0.5*s[b,k,n] = fused[b,n,:] @ wlin[:,k]
                   + 0.2*(a_q[k] . |q_T|) + 0.2*(a_k[k] . |k_T|)
    with wlin[:,k] = 0.3*(W_q[k] @ a_q[k] + W_k[k] @ a_k[k]) host-folded.
  * bias_log content is batch-independent ([K,N,N] broadcast over B).

Sharding (8 cores, SPMD, zero cross-core communication): core m owns
basis k=m for ALL batches; pbar (batch mean) is a local partition-sum.
Each batch is processed end-to-end (fused -> proj -> e -> softmax ->
broadcast -> alpha DMA) so output DMA starts ~8us in and streams
continuously — the kernel is output-bandwidth-bound as intended for
this memory-regime problem.  PE matmuls run fp32r (fp22 multiplies,
fp32 accumulate) except the p-broadcasts, which stay exact fp32.
"""

import sys

import numpy as np

if "/opt/trn_rl_repo" not in sys.path:
    sys.path.insert(0, "/opt/trn_rl_repo")

from contextlib import ExitStack

import concourse.bass as bass
import concourse.tile as tile
from concourse import bacc, mybir
from concourse.bass_utils import run_bass_kernel_spmd

B, N, D, K = 8, 512, 128, 8
R = D
MOM = 0.99
EPS = 1e-6
N_CORES = 8
F32 = mybir.dt.float32
F32R = mybir.dt.float32r
AF = mybir.ActivationFunctionType
ALU = mybir.AluOpType


def build():
    """Build the SPMD per-core Bass program (identical on all 8 cores)."""
    nc = bacc.Bacc("TRN2", target_bir_lowering=False, debug=False,
                   num_devices=N_CORES)

    # ---- per-core external tensors -------------------------------------
    # xTall[b,h,d,n]: h=0 desc[b].T, h=1 nve[b].T  (same array on all cores)
    xTall = nc.dram_tensor("xTall", [B, 2, D, N], F32R, kind="ExternalInput")
    wfuse = nc.dram_tensor("wfuse", [2, D, D], F32R, kind="ExternalInput")
    bfuse = nc.dram_tensor("bfuse", [D, 1], F32, kind="ExternalInput")
    wq = nc.dram_tensor("wq", [D, R], F32R, kind="ExternalInput")   # W_q[m]
    wk = nc.dram_tensor("wk", [D, R], F32R, kind="ExternalInput")   # W_k[m]
    aq1 = nc.dram_tensor("aq1", [R, 1], F32R, kind="ExternalInput")
    ak1 = nc.dram_tensor("ak1", [R, 1], F32R, kind="ExternalInput")
    wlin1 = nc.dram_tensor("wlin1", [D, 1], F32R, kind="ExternalInput")
    ema = nc.dram_tensor("ema", [N, N], F32, kind="ExternalInput")  # [m]
    alpha = nc.dram_tensor("alpha", [B, N, N], F32, kind="ExternalOutput")
    biaso = nc.dram_tensor("bias", [B, N, N], F32, kind="ExternalOutput")

    with ExitStack() as ctx:
        tc = ctx.enter_context(tile.TileContext(nc))
        const = ctx.enter_context(tc.tile_pool(name="const", bufs=1))
        work = ctx.enter_context(tc.tile_pool(name="work", bufs=2))
        absp = ctx.enter_context(tc.tile_pool(name="absp", bufs=4))
        psum = ctx.enter_context(tc.tile_pool(name="psum", bufs=1, space="PSUM"))

        # tiles declared up-front; loads emitted in latency-aware order
        wfuse_sb = const.tile([D, 2 * D], F32R)
        bfuse_sb = const.tile([D, 1], F32)
        wq_sb = const.tile([D, R], F32R)
        wk_sb = const.tile([D, R], F32R)
        aq_sb = const.tile([R, 1], F32R)
        ak_sb = const.tile([R, 1], F32R)
        wlin_sb = const.tile([D, 1], F32R)
        ones1_sb = const.tile([1, D], F32)
        cones8_sb = const.tile([B, 1], F32)
        rep_sb = const.tile([128, B * N], F32)
        p_all = const.tile([B, N], F32)
        ema_sb = const.tile([128, 4 * N], F32)
        bias_sb = const.tile([128, 4 * N], F32)

        # fusion weights + first batch first — they gate the first matmul
        nc.sync.dma_start(wfuse_sb[:].rearrange("d (h c) -> d h c", h=2),
                          wfuse.ap().rearrange("h d c -> d h c"))
        nc.sync.dma_start(bfuse_sb[:], bfuse[:])
        nc.vector.memset(ones1_sb[:], 1.0)
        nc.vector.memset(cones8_sb[:], 0.01 / B / MOM)

        for b in range(B):
            xb = work.tile([D, 2 * N], F32R, tag="xb", bufs=3)
            nc.sync.dma_start(
                xb[:].rearrange("d (h n) -> d h n", h=2),
                xTall[b].rearrange("h d n -> d h n"))
            if b == 0:
                nc.sync.dma_start(wq_sb[:], wq[:])
                nc.sync.dma_start(wk_sb[:], wk[:])
                nc.sync.dma_start(aq_sb[:], aq1[:])
                nc.sync.dma_start(ak_sb[:], ak1[:])
                nc.sync.dma_start(wlin_sb[:], wlin1[:])
            psum_f = psum.tile([D, N], F32, tag="mm", bufs=2)
            nc.tensor.matmul(psum_f[:], wfuse_sb[:, 0:D], xb[:, 0:N],
                             start=True, stop=False)
            nc.tensor.matmul(psum_f[:], wfuse_sb[:, D:2 * D],
                             xb[:, N:2 * N], start=False, stop=True)
            fused_sb = absp.tile([D, N], F32R, tag="fused", bufs=2)
            nc.scalar.activation(fused_sb[:], psum_f[:], AF.Identity,
                                 bias=bfuse_sb[:], scale=1.0)
            psum_s = psum.tile([1, N], F32, tag="ps", bufs=2)
            nc.tensor.matmul(psum_s[:], wlin_sb[:], fused_sb[:],
                             start=True, stop=False)
            psum_q = psum.tile([D, N], F32, tag="mm", bufs=2)
            nc.tensor.matmul(psum_q[:], wq_sb[:], fused_sb[:],
                             start=True, stop=True)
            absq = absp.tile([D, N], F32R, tag="abs", bufs=2)
            nc.scalar.activation(absq[:], psum_q[:], AF.Abs)
            nc.tensor.matmul(psum_s[:], aq_sb[:], absq[:],
                             start=False, stop=False)
            psum_k = psum.tile([D, N], F32, tag="mm", bufs=2)
            nc.tensor.matmul(psum_k[:], wk_sb[:], fused_sb[:],
                             start=True, stop=True)
            absk = absp.tile([D, N], F32R, tag="abs", bufs=2)
            nc.scalar.activation(absk[:], psum_k[:], AF.Abs)
            nc.tensor.matmul(psum_s[:], ak_sb[:], absk[:],
                             start=False, stop=True)

            # ---- softmax over free dim: p_b [1, N] ---------------------
            mx = work.tile([1, 1], F32, tag="mx", bufs=2)
            nc.vector.reduce_max(mx[:], psum_s[:], axis=mybir.AxisListType.X)
            negmx = work.tile([1, 1], F32, tag="nmx", bufs=2)
            nc.vector.tensor_scalar_mul(negmx[:], mx[:], -1.0)
            expv = work.tile([1, N], F32, tag="ex", bufs=2)
            sume = work.tile([1, 1], F32, tag="se", bufs=2)
            nc.scalar.activation(expv[:], psum_s[:], AF.Exp, bias=negmx[:],
                                 scale=1.0, accum_out=sume[:])
            rsum = work.tile([1, 1], F32, tag="rs", bufs=2)
            nc.vector.reciprocal(rsum[:], sume[:])
            p_b = work.tile([1, N], F32, tag="p", bufs=2)
            nc.vector.tensor_scalar_mul(p_b[:], expv[:], rsum[:])
            # collect p rows for pbar on the SWDGE queue (off the HWDGE path)
            nc.gpsimd.dma_start(p_all[b:b + 1, :], p_b[:])

            # ---- alpha[b, i, :] = p_b for all i ------------------------
            psum_rep = psum.tile([128, N], F32, tag="rep", bufs=2)
            nc.tensor.matmul(psum_rep[:], ones1_sb[:], p_b[:],
                             start=True, stop=True)
            nc.vector.tensor_copy(rep_sb[:, bass.ts(b, N)], psum_rep[:])
            src = rep_sb[:, bass.ts(b, N)].rearrange(
                "p (o n) -> p o n", o=1).broadcast_to([128, 4, N])
            dst = alpha[b].rearrange("(p i) j -> p i j", p=128)
            nc.sync.dma_start(dst, src)
            if b == 0:
                nc.sync.dma_start(
                    ema_sb[:].rearrange("p (c n) -> p c n", c=4),
                    ema.ap().rearrange("(c p) n -> p c n", p=128))

        # ---- bias_log: pbar is LOCAL (partition-sum over batches) ------
        psum_pb1 = psum.tile([1, N], F32, tag="rep", bufs=2)
        nc.tensor.matmul(psum_pb1[:], cones8_sb[:], p_all[:],
                         start=True, stop=True)
        pb_sb = work.tile([1, N], F32, bufs=1)
        nc.vector.tensor_copy(pb_sb[:], psum_pb1[:])
        psum_pb = psum.tile([128, N], F32, tag="rep", bufs=2)
        nc.tensor.matmul(psum_pb[:], ones1_sb[:], pb_sb[:],
                         start=True, stop=True)
        for c in range(4):
            u = work.tile([128, N], F32, tag="u", bufs=2)
            nc.vector.tensor_add(u[:], ema_sb[:, bass.ts(c, N)], psum_pb[:])
            v = work.tile([128, N], F32, tag="v", bufs=2)
            nc.vector.tensor_scalar_max(v[:], u[:], EPS / MOM)
            nc.scalar.activation(bias_sb[:, bass.ts(c, N)], v[:], AF.Ln,
                                 scale=MOM)
            src = bias_sb[:, bass.ts(c, N)].rearrange(
                "p (o n) -> p o n", o=1).broadcast_to([128, B, N])
            dst = biaso.ap().rearrange("b (c p) j -> c p b j", c=4)[c]
            nc.sync.dma_start(dst, src)

    nc.compile()
    return nc


_NC_CACHE = None


def _get_nc():
    global _NC_CACHE
    if _NC_CACHE is None:
        _NC_CACHE = build()
    return _NC_CACHE


def make_in_maps(desc_embeddings, name_value_embeddings, W_fuse, b_fuse,
                 W_q, W_k, a, alpha_ema):
    """Host-side sharding / weight prep -> per-core input dicts."""
    desc = np.asarray(desc_embeddings, np.float32)
    nve = np.asarray(name_value_embeddings, np.float32)
    W_fuse = np.asarray(W_fuse, np.float32)
    b_fuse = np.asarray(b_fuse, np.float32)
    W_q = np.asarray(W_q, np.float32)
    W_k = np.asarray(W_k, np.float32)
    a = np.asarray(a, np.float32)
    alpha_ema = np.asarray(alpha_ema, np.float32)

    a_q = a[:, :R, 0]                      # [K,R]
    a_k = a[:, R:, 0]                      # [K,R]
    wlin = 0.3 * (np.einsum("kdr,kr->kd", W_q, a_q)
                  + np.einsum("kdr,kr->kd", W_k, a_k))  # [K,D]

    # xTall[b] = [desc[b].T, nve[b].T] — shared across cores
    xTall = np.ascontiguousarray(
        np.stack([np.stack([desc[b].T, nve[b].T], axis=0)
                  for b in range(B)], axis=0))
    wfuse_stack = np.ascontiguousarray(W_fuse.reshape(2, D, D))
    bfuse_col = np.ascontiguousarray(b_fuse.reshape(D, 1))

    shared = dict(xTall=xTall, wfuse=wfuse_stack, bfuse=bfuse_col)
    in_maps = []
    for m in range(N_CORES):
        in_maps.append(dict(
            shared,
            wq=np.ascontiguousarray(W_q[m]),
            wk=np.ascontiguousarray(W_k[m]),
            aq1=np.ascontiguousarray(0.2 * a_q[m].reshape(R, 1)),
            ak1=np.ascontiguousarray(0.2 * a_k[m].reshape(R, 1)),
            wlin1=np.ascontiguousarray(wlin[m].reshape(D, 1)),
            ema=np.ascontiguousarray(alpha_ema[m])))
    return in_maps


def gather(results):
    alpha_full = np.stack([r["alpha"] for r in results], axis=1)
    bias_full = np.stack([r["bias"] for r in results], axis=1)
    return bias_full, alpha_full


def kernel(**inputs):
    nc = _get_nc()
    in_maps = make_in_maps(**inputs)
    res = run_bass_kernel_spmd(nc, in_maps, list(range(N_CORES)))
    return gather(res.results)
